# revision 19
# baseline (speedup 1.0000x reference)
"""Block-sparse (block-diagonal, BLOCK=64) multi-head attention for 8 Trainium2 cores.

Sharding: the B*S = 4096 token rows are split into 8 contiguous slices of 512
tokens (attention is block-diagonal with 64-token blocks, so slices at
512-token boundaries are fully independent). Each core runs the whole
projections + attention + output projection for its 512 tokens; weights are
replicated. No collectives; host concatenates the per-core outputs.

Layout strategy (per core):
  - host passes every tensor as a partition-contiguous [128, N] blob so each
    input DMA has one descriptor per partition (DMA *issue* on the
    sequencers, ~630ns per DIRECT2D, was a lead-in bottleneck at 68 DMAs).
    All input DMAs go on the sync queue in first-use order -- the wire runs
    at the ~330 GB/s HBM roofline for ~35us, so arrival order must match
    consumption order; xq/wq are split so the first matmul starts ~5us in.
  - Q^T, K^T are produced feature-major [dout, t] (lhsT = W tile, rhs = X^T).
  - V is produced token-major [t, dout] up front (dense N=512 matmuls).
  - attention is software-pipelined one head-pair ahead: the PE stream per
    slot is [scores(k+1) | rowsum(k) | AV(k) | y-proj MMs], so the PE never
    waits on the exp() chain -- recurring micro-idles re-throttle the HAM
    clock gate to half rate (that cost 48us in a previous revision).
  - scores: two K=64 matmuls packed into different PSUM banks + different PE
    row groups (partitions 0:64 / 64:128) so they run concurrently. exp() of
    the four diagonal 64x64 quadrants takes two scalar-engine calls with 3D
    access patterns; the rest of the P tile is zeroed by a GpSimd memset.
  - row sums r[i]: ones-vector matmul pair (col-group packed) replicates the
    in-block column sums into the two 64-partition halves of one PSUM tile;
    reciprocal_approx_fast gives 1/r, and normalization is folded into the
    PSUM->SBUF copy of the attention output (tensor_mul).
  - O^T[dv, i] = V.T @ P (col-group packed pair) is feature-major, directly
    the lhsT of the output projection. Output-projection partials are
    scheduled into the attention loop as soon as their oT tiles exist
    (stagger by head pair), so the post-loop tail is only the last two
    partials + bias + DMA.

Compute dtype: fp16 operands with fp32 PSUM accumulation; y is written fp16
(host casts back). The all-f32 variant is available via _compute="f32".
"""

import sys

sys.path.insert(0, "/opt/trn_rl_repo")

import numpy as np

N_CORES = 8
B, S, D = 2, 2048, 1024
H, DK = 16, 64
T = (B * S) // N_CORES      # 512 tokens per core
P = 128
KO = D // P                 # 8 contraction tiles
MO = D // P                 # 8 d_out tiles
NC_CHUNKS = T // P          # 4 token chunks per core
HP = H // 2                 # 8 head pairs
NV = D // T                 # 2 output column halves of 512
N_WARM = 130                # junk matmuls covering the DMA lead-in

_cache = {}


def _build_program(compute):
    import concourse.tile as tile
    from concourse import bacc, mybir

    f32 = mybir.dt.float32
    dtc = {"f32": f32, "f16": mybir.dt.float16, "bf16": mybir.dt.bfloat16}[compute]

    nc = bacc.Bacc("TRN2", target_bir_lowering=False, debug=False)

    xq_d = nc.dram_tensor("xq", [P, KO * T], dtc, kind="ExternalInput").ap()
    xk_d = nc.dram_tensor("xk", [P, KO * T], dtc, kind="ExternalInput").ap()
    xv_d = nc.dram_tensor("xv", [P, KO * T], dtc, kind="ExternalInput").ap()
    wq_d = nc.dram_tensor("wq", [P, MO * KO * P], dtc, kind="ExternalInput").ap()
    wk_d = nc.dram_tensor("wk", [P, MO * KO * P], dtc, kind="ExternalInput").ap()
    wv_d = nc.dram_tensor("wv", [P, KO * D], dtc, kind="ExternalInput").ap()
    wo_d = nc.dram_tensor("wo", [P, MO * D], dtc, kind="ExternalInput").ap()
    bqk_d = nc.dram_tensor("bqk", [P, 2 * MO], f32, kind="ExternalInput").ap()
    bv_d = nc.dram_tensor("bv", [D], f32, kind="ExternalInput").ap()
    y_d = nc.dram_tensor("y", [T, D], dtc, kind="ExternalOutput").ap()

    with tile.TileContext(nc) as tc:
        with (
            tc.tile_pool(name="singles", bufs=1) as singles,
            tc.tile_pool(name="p2", bufs=4) as p2_pool,
            tc.tile_pool(name="rec", bufs=3) as rec_pool,
            tc.tile_pool(name="ystage", bufs=3) as y_pool,
            tc.tile_pool(name="psbig", bufs=2, space="PSUM") as ps_big,
            tc.tile_pool(name="pss", bufs=2, space="PSUM") as pss_pool,
            tc.tile_pool(name="psro", bufs=2, space="PSUM") as psro_pool,
        ):
            # ---- persistent SBUF tensors ----
            xq_sb = singles.tile([P, KO, T], dtc, tag="xq")
            xk_sb = singles.tile([P, KO, T], dtc, tag="xk")
            xv_sb = singles.tile([P, KO, T], dtc, tag="xv")
            wq_sb = singles.tile([P, MO, KO, P], dtc, tag="wq")
            wk_sb = singles.tile([P, MO, KO, P], dtc, tag="wk")
            wv_sb = singles.tile([P, KO, D], dtc, tag="wv")
            wo_sb = singles.tile([P, MO, D], dtc, tag="wo")
            qT_sb = singles.tile([P, MO, T], dtc, tag="qT")
            kT_sb = singles.tile([P, MO, T], dtc, tag="kT")
            v_sb = singles.tile([P, NC_CHUNKS, D], dtc, tag="v")
            oT_sb = singles.tile([P, MO, T], dtc, tag="oT")
            bqk_sb = singles.tile([P, 2 * MO], f32, tag="bqk")
            bv_sb = singles.tile([P, D], f32, tag="bv")
            ones_sb = singles.tile([P, P], dtc, tag="ones")
            scratch_sb = singles.tile([P, T], dtc, tag="scratch")

            # PE warm-up: junk matmuls during the DMA lead-in so the HAM
            # clock gate is at full rate when the projections start
            nc.vector.memset(ones_sb[:], 1.0)
            ps_w = psro_pool.tile([P, T], f32, tag="psro", name="warmup")
            for _ in range(N_WARM):
                nc.tensor.matmul(ps_w[0:64, 0:P], ones_sb[:, 0:64],
                                 ones_sb[:], start=True, stop=True)

            # input DMAs on TWO parallel HWDGE queues (sync: activations,
            # scalar: weights+biases) so the per-DMA completion-receipt
            # stalls overlap the other queue's streaming; within each queue
            # strict first-use order. First matmul needs only xqA + wq m0-1.
            xq_flat = xq_sb[:].rearrange("p ko t -> p (ko t)")
            wq_flat = wq_sb[:].rearrange("p m ko c -> p (m ko c)")
            wk_flat = wk_sb[:].rearrange("p m ko c -> p (m ko c)")
            hx = KO * T // 2
            half = MO // 2 * KO * P
            nc.sync.dma_start(xq_flat[:], xq_d[:])
            nc.sync.dma_start(xk_sb[:].rearrange("p ko t -> p (ko t)"), xk_d[:])
            nc.sync.dma_start(wk_flat[:, half:], wk_d[:, half:])
            nc.scalar.dma_start(wq_flat[:, 0:half], wq_d[:, 0:half])
            nc.scalar.dma_start(bqk_sb[:], bqk_d[:])
            nc.scalar.dma_start(wq_flat[:, half:], wq_d[:, half:])
            nc.scalar.dma_start(wk_flat[:, 0:half], wk_d[:, 0:half])
            # third stream (SWDGE) for the tensors needed late; dummy memsets
            # delay its first descriptor so it doesn't steal early HBM
            # bandwidth from the Q/K-critical pieces above
            for _ in range(28):
                nc.gpsimd.memset(scratch_sb[:], 0.0)
            nc.gpsimd.dma_start(xv_sb[:].rearrange("p ko t -> p (ko t)"), xv_d[:])
            nc.gpsimd.dma_start(wv_sb[:].rearrange("p ko d -> p (ko d)"), wv_d[:])
            nc.gpsimd.dma_start(wo_sb[:].rearrange("p m d -> p (m d)"), wo_d[:])
            nc.gpsimd.dma_start(bv_sb[:], bv_d[None, :].to_broadcast([P, D]))

            # ---- Q^T / K^T projections (feature-major out) ----
            for w_sb, x_sb, boff, dst in (
                (wq_sb, xq_sb, 0, qT_sb),
                (wk_sb, xk_sb, MO, kT_sb),
            ):
                for m in range(MO):
                    ps = ps_big.tile([P, T], f32, tag="psbig")
                    for ko in range(KO):
                        nc.tensor.matmul(
                            ps[:],
                            w_sb[:, m, ko, :],
                            x_sb[:, ko, :],
                            start=(ko == 0),
                            stop=(ko == KO - 1),
                        )
                    nc.scalar.activation(
                        dst[:, m, :],
                        ps[:],
                        mybir.ActivationFunctionType.Identity,
                        bias=bqk_sb[:, boff + m : boff + m + 1],
                    )

            # ---- V projection, token-major, all chunks up front ----
            for mt in range(NC_CHUNKS):
                for n in range(NV):
                    ps = ps_big.tile([P, T], f32, tag="psbig")
                    for ko in range(KO):
                        nc.tensor.matmul(
                            ps[:],
                            xv_sb[:, ko, mt * P : (mt + 1) * P],
                            wv_sb[:, ko, n * T : (n + 1) * T],
                            start=(ko == 0),
                            stop=(ko == KO - 1),
                        )
                    nc.vector.tensor_add(
                        v_sb[:, mt, n * T : (n + 1) * T],
                        ps[:],
                        bv_sb[:, n * T : (n + 1) * T],
                    )

            # ---- output projection, scheduled into the attention loop ----
            y_ps = [[None] * NV for _ in range(NC_CHUNKS)]

            def y_proj_part(c, j):
                # accumulate m-pair (2j, 2j+1) of both column halves for
                # token chunk c; m-major order so consecutive matmuls share
                # the stationary operand (LDWEIGHTS amortized), and the 4
                # N=512 matmuls run as one dense burst
                if j == 0:
                    for n in range(NV):
                        y_ps[c][n] = ps_big.tile([P, T], f32, tag="psbig",
                                                 name=f"psy_{c}_{n}")
                for m in (2 * j, 2 * j + 1):
                    for n in range(NV):
                        nc.tensor.matmul(
                            y_ps[c][n][:],
                            oT_sb[:, m, c * P : (c + 1) * P],
                            wo_sb[:, m, n * T : (n + 1) * T],
                            start=(m == 0),
                            stop=(m == MO - 1),
                        )
                if j == 3:
                    # bias bo is added on the host; evacuation is a pure
                    # fp32->fp16 copy. The very last half goes on the scalar
                    # engine so both final halves evacuate in parallel.
                    for n in range(NV):
                        y_sb = y_pool.tile([P, T], dtc, tag="ystage")
                        if c == NC_CHUNKS - 1 and n == 1:
                            nc.scalar.copy(y_sb[:], y_ps[c][n][:])
                            nc.scalar.dma_start(
                                y_d[c * P : (c + 1) * P, n * T : (n + 1) * T],
                                y_sb[:],
                            )
                        else:
                            nc.vector.tensor_copy(y_sb[:], y_ps[c][n][:])
                            nc.sync.dma_start(
                                y_d[c * P : (c + 1) * P, n * T : (n + 1) * T],
                                y_sb[:],
                            )
                        y_ps[c][n] = None

            # y part (c, j) consumes oT tiles (c, 2j) and (c, 2j+1);
            # schedule it at global slot c*8 + 2j + 2 (dep-satisfied).
            y_sched = {}
            for c in range(NC_CHUNKS):
                for j in range(4):
                    y_sched.setdefault(c * HP + 2 * j + 2, []).append((c, j))

            # ---- attention, software-pipelined one head pair ahead ----
            def scores(k):
                # K=64 pair in different PSUM banks (cols 0:128 vs 512:640)
                # and different PE row groups -> runs concurrently
                c, hp = k // HP, k % HP
                tsl = slice(c * P, (c + 1) * P)
                ps = pss_pool.tile([P, 2 * T], f32, tag="pss")
                nc.tensor.matmul(
                    ps[:, 0:P],
                    kT_sb[0:64, hp, tsl],
                    qT_sb[0:64, hp, tsl],
                    start=True, stop=True,
                )
                nc.tensor.matmul(
                    ps[:, T : T + P],
                    kT_sb[64:128, hp, tsl],
                    qT_sb[64:128, hp, tsl],
                    start=True, stop=True,
                )
                return ps

            n_iters = NC_CHUNKS * HP
            ps_cur = scores(0)
            for k in range(n_iters + 2):
                for item in y_sched.get(k, ()):
                    y_proj_part(*item)
                if k < n_iters:
                    c, hp = k // HP, k % HP
                    tsl = slice(c * P, (c + 1) * P)
                    h0, h1 = 2 * hp, 2 * hp + 1

                    p2 = p2_pool.tile([P, 2 * P], dtc, tag="p2")
                    nc.gpsimd.memset(p2[:], 0.0)
                    # exp() of the diagonal quadrants of both heads: two
                    # 3D-AP calls (head dim as middle loop, stride T)
                    psg = ps_cur[:].rearrange("p (g f) -> p g f", g=2)
                    p2g = p2[:].rearrange("p (g f) -> p g f", g=2)
                    nc.scalar.activation(
                        p2g[0:64, :, 0:64],
                        psg[0:64, :, 0:64],
                        mybir.ActivationFunctionType.Exp,
                        scale=0.125,
                    )
                    nc.scalar.activation(
                        p2g[64:128, :, 64:128],
                        psg[64:128, :, 64:128],
                        mybir.ActivationFunctionType.Exp,
                        scale=0.125,
                    )
                    # PE: next slot's scores first (independent of exp)
                    if k + 1 < n_iters:
                        ps_next = scores(k + 1)
                    # replicated in-block column sums (col-group packed)
                    ps_r = psro_pool.tile([P, T], f32, tag="psro")
                    nc.tensor.matmul(
                        ps_r[0:64, 0:P], ones_sb[:, 0:64], p2[:, 0:P],
                        start=True, stop=True,
                    )
                    nc.tensor.matmul(
                        ps_r[64:128, 0:P], ones_sb[:, 0:64], p2[:, P : 2 * P],
                        start=True, stop=True,
                    )
                    rec = rec_pool.tile([P, P], f32, tag="rec")
                    nc.vector.reciprocal_approx_fast(out=rec[:], in_=ps_r[:, 0:P])
                    # attention output (col-group packed pair)
                    ps_o = psro_pool.tile([P, T], f32, tag="psro")
                    nc.tensor.matmul(
                        ps_o[0:64, 0:P],
                        v_sb[:, c, h0 * DK : (h0 + 1) * DK],
                        p2[:, 0:P],
                        start=True, stop=True,
                    )
                    nc.tensor.matmul(
                        ps_o[64:128, 0:P],
                        v_sb[:, c, h1 * DK : (h1 + 1) * DK],
                        p2[:, P : 2 * P],
                        start=True, stop=True,
                    )
                    nc.vector.tensor_mul(oT_sb[:, hp, tsl], ps_o[:, 0:P], rec[:])
                    ps_cur = ps_next if k + 1 < n_iters else None

    nc.compile()
    return nc


def _get_program(compute):
    if compute not in _cache:
        _cache[compute] = _build_program(compute)
    return _cache[compute]


DEFAULT_COMPUTE = "f16"


def kernel(
    query,
    key,
    value,
    Wq,
    bq,
    Wk,
    bk,
    Wv,
    bv,
    Wo,
    bo,
    _compute=DEFAULT_COMPUTE,
    _trace=False,
):
    from concourse.bass_utils import run_bass_kernel_spmd

    nc = _get_program(_compute)
    if _compute == "bf16":
        import ml_dtypes

        npdt = ml_dtypes.bfloat16
    else:
        npdt = {"f32": np.float32, "f16": np.float16}[_compute]

    def pre_wqk(w):
        # [din, dout] -> [c, m, ko, p'] partition-contiguous blob
        return np.ascontiguousarray(
            np.asarray(w, np.float32)
            .reshape(KO, P, MO, P)
            .transpose(1, 2, 0, 3)
            .reshape(P, MO * KO * P)
            .astype(npdt)
        )

    def pre_wrow(w):
        # [din, dout] -> [c, ko, dout]: partition c holds its KO din-rows
        return np.ascontiguousarray(
            np.asarray(w, np.float32)
            .reshape(KO, P, D)
            .transpose(1, 0, 2)
            .reshape(P, KO * D)
            .astype(npdt)
        )

    def pre_x(x2, rows):
        # [t, d] slice -> feature-major [c, ko, t] blob
        return np.ascontiguousarray(
            x2[rows].T.reshape(KO, P, T).transpose(1, 0, 2).reshape(P, KO * T)
            .astype(npdt)
        )

    bqk = np.concatenate(
        [
            np.asarray(bq, np.float32).reshape(MO, P).T,
            np.asarray(bk, np.float32).reshape(MO, P).T,
        ],
        axis=1,
    )

    q2 = np.asarray(query, np.float32).reshape(B * S, D)
    k2 = np.asarray(key, np.float32).reshape(B * S, D)
    v2 = np.asarray(value, np.float32).reshape(B * S, D)
    shared = {
        "wq": pre_wqk(Wq),
        "wk": pre_wqk(Wk),
        "wv": pre_wrow(Wv),
        "wo": pre_wrow(Wo),
        "bqk": np.ascontiguousarray(bqk),
        "bv": np.ascontiguousarray(np.asarray(bv, np.float32)),
    }
    in_maps = []
    for c in range(N_CORES):
        rows = slice(c * T, (c + 1) * T)
        in_maps.append(
            {
                "xq": pre_x(q2, rows),
                "xk": pre_x(k2, rows),
                "xv": pre_x(v2, rows),
                **shared,
            }
        )

    kwargs = {}
    if _trace:
        kwargs = {"trace": True}
    res = run_bass_kernel_spmd(nc, in_maps, core_ids=list(range(N_CORES)), **kwargs)
    y = np.concatenate(
        [np.asarray(res.results[c]["y"], np.float32) for c in range(N_CORES)],
        axis=0,
    )
    y += np.asarray(bo, np.float32)  # output bias folded in on the host
    out = y.reshape(B, S, D)
    if _trace:
        return out, res
    return out


# revision 23
# speedup vs baseline: 1.1294x; 1.1294x over previous
"""Block-sparse (block-diagonal, BLOCK=64) multi-head attention for 8 Trainium2 cores.

Sharding: the B*S = 4096 token rows are split into 8 contiguous slices of 512
tokens (attention is block-diagonal with 64-token blocks, so slices at
512-token boundaries are fully independent). Each core runs the whole
projections + attention + output projection for its 512 tokens; weights are
replicated. No collectives; host concatenates the per-core outputs.

Layout strategy (per core):
  - host passes every tensor as a partition-contiguous [128, N] blob so each
    input DMA has one descriptor per partition (DMA *issue* on the
    sequencers, ~630ns per DIRECT2D, was a lead-in bottleneck at 68 DMAs).
    All input DMAs go on the sync queue in first-use order -- the wire runs
    at the ~330 GB/s HBM roofline for ~35us, so arrival order must match
    consumption order; xq/wq are split so the first matmul starts ~5us in.
  - Q^T, K^T are produced feature-major [dout, t] (lhsT = W tile, rhs = X^T).
  - V is produced token-major [t, dout] up front (dense N=512 matmuls).
  - attention is software-pipelined one head-pair ahead: the PE stream per
    slot is [scores(k+1) | rowsum(k) | AV(k) | y-proj MMs], so the PE never
    waits on the exp() chain -- recurring micro-idles re-throttle the HAM
    clock gate to half rate (that cost 48us in a previous revision).
  - scores: two K=64 matmuls packed into different PSUM banks + different PE
    row groups (partitions 0:64 / 64:128) so they run concurrently. exp() of
    the four diagonal 64x64 quadrants takes two scalar-engine calls with 3D
    access patterns; the rest of the P tile is zeroed by a GpSimd memset.
  - row sums r[i]: ones-vector matmul pair (col-group packed) replicates the
    in-block column sums into the two 64-partition halves of one PSUM tile;
    reciprocal_approx_fast gives 1/r, and normalization is folded into the
    PSUM->SBUF copy of the attention output (tensor_mul).
  - O^T[dv, i] = V.T @ P (col-group packed pair) is feature-major, directly
    the lhsT of the output projection. Output-projection partials are
    scheduled into the attention loop as soon as their oT tiles exist
    (stagger by head pair), so the post-loop tail is only the last two
    partials + bias + DMA.

Compute dtype: fp16 operands with fp32 PSUM accumulation; y is written fp16
(host casts back). The all-f32 variant is available via _compute="f32".
"""

import sys

sys.path.insert(0, "/opt/trn_rl_repo")

import numpy as np

N_CORES = 8
B, S, D = 2, 2048, 1024
H, DK = 16, 64
T = (B * S) // N_CORES      # 512 tokens per core
P = 128
KO = D // P                 # 8 contraction tiles
MO = D // P                 # 8 d_out tiles
NC_CHUNKS = T // P          # 4 token chunks per core
HP = H // 2                 # 8 head pairs
NV = D // T                 # 2 output column halves of 512
N_WARM = 130                # junk matmuls covering the DMA lead-in

_cache = {}


def _build_program(compute):
    import concourse.tile as tile
    from concourse import bacc, mybir

    f32 = mybir.dt.float32
    dtc = {"f32": f32, "f16": mybir.dt.float16, "bf16": mybir.dt.bfloat16}[compute]

    nc = bacc.Bacc("TRN2", target_bir_lowering=False, debug=False)

    xq_d = nc.dram_tensor("xq", [P, KO * T], dtc, kind="ExternalInput").ap()
    xk_d = nc.dram_tensor("xk", [P, KO * T], dtc, kind="ExternalInput").ap()
    xv_d = nc.dram_tensor("xv", [P, KO * T], dtc, kind="ExternalInput").ap()
    wq_d = nc.dram_tensor("wq", [P, MO * KO * P], dtc, kind="ExternalInput").ap()
    wk_d = nc.dram_tensor("wk", [P, MO * KO * P], dtc, kind="ExternalInput").ap()
    wv_d = nc.dram_tensor("wv", [P, KO * D], dtc, kind="ExternalInput").ap()
    wo_d = nc.dram_tensor("wo", [P, MO * D], dtc, kind="ExternalInput").ap()
    bqk_d = nc.dram_tensor("bqk", [P, 2 * MO], f32, kind="ExternalInput").ap()
    bv_d = nc.dram_tensor("bv", [D], f32, kind="ExternalInput").ap()
    y_d = nc.dram_tensor("y", [T, D], dtc, kind="ExternalOutput").ap()

    with tile.TileContext(nc) as tc:
        with (
            tc.tile_pool(name="singles", bufs=1) as singles,
            tc.tile_pool(name="p2", bufs=4) as p2_pool,
            tc.tile_pool(name="rec", bufs=3) as rec_pool,
            tc.tile_pool(name="ystage", bufs=3) as y_pool,
            tc.tile_pool(name="psbig", bufs=2, space="PSUM") as ps_big,
            tc.tile_pool(name="pss", bufs=2, space="PSUM") as pss_pool,
            tc.tile_pool(name="psro", bufs=2, space="PSUM") as psro_pool,
        ):
            # ---- persistent SBUF tensors ----
            xq_sb = singles.tile([P, KO, T], dtc, tag="xq")
            xk_sb = singles.tile([P, KO, T], dtc, tag="xk")
            xv_sb = singles.tile([P, KO, T], dtc, tag="xv")
            wq_sb = singles.tile([P, MO, KO, P], dtc, tag="wq")
            wk_sb = singles.tile([P, MO, KO, P], dtc, tag="wk")
            wv_sb = singles.tile([P, KO, D], dtc, tag="wv")
            wo_sb = singles.tile([P, MO, D], dtc, tag="wo")
            qT_sb = singles.tile([P, MO, T], dtc, tag="qT")
            kT_sb = singles.tile([P, MO, T], dtc, tag="kT")
            v_sb = singles.tile([P, NC_CHUNKS, D], dtc, tag="v")
            oT_sb = singles.tile([P, MO, T], dtc, tag="oT")
            bqk_sb = singles.tile([P, 2 * MO], f32, tag="bqk")
            bv_sb = singles.tile([P, D], f32, tag="bv")
            ones_sb = singles.tile([P, P], dtc, tag="ones")
            scratch_sb = singles.tile([P, T], dtc, tag="scratch")

            # PE warm-up: junk matmuls during the DMA lead-in so the HAM
            # clock gate is at full rate when the projections start
            nc.vector.memset(ones_sb[:], 1.0)
            ps_w = psro_pool.tile([P, T], f32, tag="psro", name="warmup")
            for _ in range(N_WARM):
                nc.tensor.matmul(ps_w[0:64, 0:P], ones_sb[:, 0:64],
                                 ones_sb[:], start=True, stop=True)

            # input DMAs on TWO parallel HWDGE queues (sync: activations,
            # scalar: weights+biases) so the per-DMA completion-receipt
            # stalls overlap the other queue's streaming; within each queue
            # strict first-use order. First matmul needs only xqA + wq m0-1.
            xq_flat = xq_sb[:].rearrange("p ko t -> p (ko t)")
            wq_flat = wq_sb[:].rearrange("p m ko c -> p (m ko c)")
            wk_flat = wk_sb[:].rearrange("p m ko c -> p (m ko c)")
            hx = KO * T // 2
            # two HWDGE rings; pieces sized/ordered from measured ring
            # cadence (stream at ~165 GB/s while both busy + ~1us per-piece
            # receipt stall) so no consumer ever waits longer than the 3.4us
            # HAM re-throttle window
            q14 = MO // 4 * KO * P
            nc.sync.dma_start(xq_flat[:, 0:hx], xq_d[:, 0:hx])
            nc.sync.dma_start(xq_flat[:, hx:], xq_d[:, hx:])
            nc.sync.dma_start(xk_sb[:].rearrange("p ko t -> p (ko t)"), xk_d[:])
            nc.sync.dma_start(wk_flat[:, 2 * q14 :], wk_d[:, 2 * q14 :])
            nc.sync.dma_start(xv_sb[:].rearrange("p ko t -> p (ko t)"), xv_d[:])
            nc.sync.dma_start(wo_sb[:].rearrange("p m d -> p (m d)"), wo_d[:])
            nc.scalar.dma_start(wq_flat[:, 0:q14], wq_d[:, 0:q14])
            nc.scalar.dma_start(bqk_sb[:], bqk_d[:])
            nc.scalar.dma_start(wq_flat[:, q14 : 2 * q14], wq_d[:, q14 : 2 * q14])
            nc.scalar.dma_start(wq_flat[:, 2 * q14 :], wq_d[:, 2 * q14 :])
            nc.scalar.dma_start(wk_flat[:, 0:q14], wk_d[:, 0:q14])
            nc.scalar.dma_start(wk_flat[:, q14 : 2 * q14], wk_d[:, q14 : 2 * q14])
            nc.scalar.dma_start(wv_sb[:].rearrange("p ko d -> p (ko d)"), wv_d[:])
            nc.scalar.dma_start(bv_sb[:], bv_d[None, :].to_broadcast([P, D]))

            # ---- Q^T / K^T projections (feature-major out) ----
            for w_sb, x_sb, boff, dst in (
                (wq_sb, xq_sb, 0, qT_sb),
                (wk_sb, xk_sb, MO, kT_sb),
            ):
                for m in range(MO):
                    ps = ps_big.tile([P, T], f32, tag="psbig")
                    for ko in range(KO):
                        nc.tensor.matmul(
                            ps[:],
                            w_sb[:, m, ko, :],
                            x_sb[:, ko, :],
                            start=(ko == 0),
                            stop=(ko == KO - 1),
                        )
                    nc.scalar.activation(
                        dst[:, m, :],
                        ps[:],
                        mybir.ActivationFunctionType.Identity,
                        bias=bqk_sb[:, boff + m : boff + m + 1],
                    )

            # ---- V projection, token-major, all chunks up front ----
            for mt in range(NC_CHUNKS):
                for n in range(NV):
                    ps = ps_big.tile([P, T], f32, tag="psbig")
                    for ko in range(KO):
                        nc.tensor.matmul(
                            ps[:],
                            xv_sb[:, ko, mt * P : (mt + 1) * P],
                            wv_sb[:, ko, n * T : (n + 1) * T],
                            start=(ko == 0),
                            stop=(ko == KO - 1),
                        )
                    nc.vector.tensor_add(
                        v_sb[:, mt, n * T : (n + 1) * T],
                        ps[:],
                        bv_sb[:, n * T : (n + 1) * T],
                    )

            # ---- output projection, scheduled into the attention loop ----
            y_ps = [[None] * NV for _ in range(NC_CHUNKS)]

            def y_proj_part(c, j):
                # accumulate m-pair (2j, 2j+1) of both column halves for
                # token chunk c; m-major order so consecutive matmuls share
                # the stationary operand (LDWEIGHTS amortized), and the 4
                # N=512 matmuls run as one dense burst
                if j == 0:
                    for n in range(NV):
                        y_ps[c][n] = ps_big.tile([P, T], f32, tag="psbig",
                                                 name=f"psy_{c}_{n}")
                for m in (2 * j, 2 * j + 1):
                    for n in range(NV):
                        nc.tensor.matmul(
                            y_ps[c][n][:],
                            oT_sb[:, m, c * P : (c + 1) * P],
                            wo_sb[:, m, n * T : (n + 1) * T],
                            start=(m == 0),
                            stop=(m == MO - 1),
                        )
                if j == 3:
                    # bias bo is added on the host; evacuation is a pure
                    # fp32->fp16 copy, split across DVE (n=0) and the scalar
                    # engine (n=1) so the two halves evacuate in parallel
                    for n in range(NV):
                        y_sb = y_pool.tile([P, T], dtc, tag="ystage")
                        if n == 1:
                            nc.scalar.copy(y_sb[:], y_ps[c][n][:])
                            nc.scalar.dma_start(
                                y_d[c * P : (c + 1) * P, n * T : (n + 1) * T],
                                y_sb[:],
                            )
                        else:
                            nc.vector.tensor_copy(y_sb[:], y_ps[c][n][:])
                            nc.sync.dma_start(
                                y_d[c * P : (c + 1) * P, n * T : (n + 1) * T],
                                y_sb[:],
                            )
                        y_ps[c][n] = None

            # y part (c, j) consumes oT tiles (c, 2j) and (c, 2j+1);
            # schedule it at global slot c*8 + 2j + 2 (dep-satisfied).
            y_sched = {}
            for c in range(NC_CHUNKS):
                for j in range(4):
                    y_sched.setdefault(c * HP + 2 * j + 2, []).append((c, j))

            # ---- attention, software-pipelined one head pair ahead ----
            def scores(k):
                # K=64 pair in different PSUM banks (cols 0:128 vs 512:640)
                # and different PE row groups -> runs concurrently
                c, hp = k // HP, k % HP
                tsl = slice(c * P, (c + 1) * P)
                ps = pss_pool.tile([P, 2 * T], f32, tag="pss")
                nc.tensor.matmul(
                    ps[:, 0:P],
                    kT_sb[0:64, hp, tsl],
                    qT_sb[0:64, hp, tsl],
                    start=True, stop=True,
                )
                nc.tensor.matmul(
                    ps[:, T : T + P],
                    kT_sb[64:128, hp, tsl],
                    qT_sb[64:128, hp, tsl],
                    start=True, stop=True,
                )
                return ps

            n_iters = NC_CHUNKS * HP
            ps_cur = scores(0)
            for k in range(n_iters + 2):
                for item in y_sched.get(k, ()):
                    y_proj_part(*item)
                if k < n_iters:
                    c, hp = k // HP, k % HP
                    tsl = slice(c * P, (c + 1) * P)
                    h0, h1 = 2 * hp, 2 * hp + 1

                    p2 = p2_pool.tile([P, 2 * P], dtc, tag="p2")
                    nc.gpsimd.memset(p2[:], 0.0)
                    # exp() of the diagonal quadrants of both heads: two
                    # 3D-AP calls (head dim as middle loop, stride T)
                    psg = ps_cur[:].rearrange("p (g f) -> p g f", g=2)
                    p2g = p2[:].rearrange("p (g f) -> p g f", g=2)
                    nc.scalar.activation(
                        p2g[0:64, :, 0:64],
                        psg[0:64, :, 0:64],
                        mybir.ActivationFunctionType.Exp,
                        scale=0.125,
                    )
                    nc.scalar.activation(
                        p2g[64:128, :, 64:128],
                        psg[64:128, :, 64:128],
                        mybir.ActivationFunctionType.Exp,
                        scale=0.125,
                    )
                    # PE: next slot's scores first (independent of exp)
                    if k + 1 < n_iters:
                        ps_next = scores(k + 1)
                    # replicated in-block column sums (col-group packed)
                    ps_r = psro_pool.tile([P, T], f32, tag="psro")
                    nc.tensor.matmul(
                        ps_r[0:64, 0:P], ones_sb[:, 0:64], p2[:, 0:P],
                        start=True, stop=True,
                    )
                    nc.tensor.matmul(
                        ps_r[64:128, 0:P], ones_sb[:, 0:64], p2[:, P : 2 * P],
                        start=True, stop=True,
                    )
                    rec = rec_pool.tile([P, P], f32, tag="rec")
                    nc.vector.reciprocal_approx_fast(out=rec[:], in_=ps_r[:, 0:P])
                    # attention output (col-group packed pair)
                    ps_o = psro_pool.tile([P, T], f32, tag="psro")
                    nc.tensor.matmul(
                        ps_o[0:64, 0:P],
                        v_sb[:, c, h0 * DK : (h0 + 1) * DK],
                        p2[:, 0:P],
                        start=True, stop=True,
                    )
                    nc.tensor.matmul(
                        ps_o[64:128, 0:P],
                        v_sb[:, c, h1 * DK : (h1 + 1) * DK],
                        p2[:, P : 2 * P],
                        start=True, stop=True,
                    )
                    nc.vector.tensor_mul(oT_sb[:, hp, tsl], ps_o[:, 0:P], rec[:])
                    ps_cur = ps_next if k + 1 < n_iters else None

    nc.compile()
    return nc


def _get_program(compute):
    if compute not in _cache:
        _cache[compute] = _build_program(compute)
    return _cache[compute]


DEFAULT_COMPUTE = "f16"


def kernel(
    query,
    key,
    value,
    Wq,
    bq,
    Wk,
    bk,
    Wv,
    bv,
    Wo,
    bo,
    _compute=DEFAULT_COMPUTE,
    _trace=False,
):
    from concourse.bass_utils import run_bass_kernel_spmd

    nc = _get_program(_compute)
    if _compute == "bf16":
        import ml_dtypes

        npdt = ml_dtypes.bfloat16
    else:
        npdt = {"f32": np.float32, "f16": np.float16}[_compute]

    def pre_wqk(w):
        # [din, dout] -> [c, m, ko, p'] partition-contiguous blob
        return np.ascontiguousarray(
            np.asarray(w, np.float32)
            .reshape(KO, P, MO, P)
            .transpose(1, 2, 0, 3)
            .reshape(P, MO * KO * P)
            .astype(npdt)
        )

    def pre_wrow(w):
        # [din, dout] -> [c, ko, dout]: partition c holds its KO din-rows
        return np.ascontiguousarray(
            np.asarray(w, np.float32)
            .reshape(KO, P, D)
            .transpose(1, 0, 2)
            .reshape(P, KO * D)
            .astype(npdt)
        )

    def pre_x(x2, rows):
        # [t, d] slice -> feature-major [c, ko, t] blob
        return np.ascontiguousarray(
            x2[rows].T.reshape(KO, P, T).transpose(1, 0, 2).reshape(P, KO * T)
            .astype(npdt)
        )

    bqk = np.concatenate(
        [
            np.asarray(bq, np.float32).reshape(MO, P).T,
            np.asarray(bk, np.float32).reshape(MO, P).T,
        ],
        axis=1,
    )

    q2 = np.asarray(query, np.float32).reshape(B * S, D)
    k2 = np.asarray(key, np.float32).reshape(B * S, D)
    v2 = np.asarray(value, np.float32).reshape(B * S, D)
    shared = {
        "wq": pre_wqk(Wq),
        "wk": pre_wqk(Wk),
        "wv": pre_wrow(Wv),
        "wo": pre_wrow(Wo),
        "bqk": np.ascontiguousarray(bqk),
        "bv": np.ascontiguousarray(np.asarray(bv, np.float32)),
    }
    in_maps = []
    for c in range(N_CORES):
        rows = slice(c * T, (c + 1) * T)
        in_maps.append(
            {
                "xq": pre_x(q2, rows),
                "xk": pre_x(k2, rows),
                "xv": pre_x(v2, rows),
                **shared,
            }
        )

    kwargs = {}
    if _trace:
        kwargs = {"trace": True}
    res = run_bass_kernel_spmd(nc, in_maps, core_ids=list(range(N_CORES)), **kwargs)
    y = np.concatenate(
        [np.asarray(res.results[c]["y"], np.float32) for c in range(N_CORES)],
        axis=0,
    )
    y += np.asarray(bo, np.float32)  # output bias folded in on the host
    out = y.reshape(B, S, D)
    if _trace:
        return out, res
    return out
